# revision 25
# baseline (speedup 1.0000x reference)
"""ASTCRFCell Trainium2 kernel: 8-core data-parallel over batch.

Math (per batch b):
  inp = [x|state] (N,66); A = E E^T (sym, batch-indep); sup = softmax(relu(A),1)
  gate sagc -> z_r = sigmoid(...); cand = [x | z*state]; hc = tanh(upd sagc)
  h = r*state + (1-r)*hc
  deg_c = softmax((A>0)*H H^T @ Wc, -1); deg_d likewise with (A<0), Wd
  out = 0.8 h - 0.1 deg_c@h + 0.1 deg_d@h

Layout strategy (per core, 2 batches):
  - supports^T computed once (column softmax via exp + PE ones-reduce,
    deferred normalization), stored bf16 [128,16,2048].
  - per batch: masked M = mask*HHT bf16 SBUF-resident [128,16,2048];
    R^T = (W^T M) via PE with W column-slices streamed from HBM;
    softmax over partition dim via exp (no max-sub; |logit|<14 verified)
    + ones-matmul colsum; deg@h accumulated in PSUM over k-tiles.
"""

import numpy as np
from contextlib import ExitStack

P = 128
N = 2048
NT = N // P          # 16
B = 16
NCORES = 8
BL = B // NCORES     # 2
CIN = 2
D = 64
DIN = CIN + D        # 66
ED = 16              # embed dim


def _build_program():
    import concourse.bass as bass
    import concourse.bacc as bacc
    import concourse.tile as tile
    from concourse import mybir
    from concourse.masks import make_identity

    fp32 = mybir.dt.float32
    bf16 = mybir.dt.float16  # fp16: 8x better mantissa than bf16, same PE speed
    Alu = mybir.AluOpType
    Act = mybir.ActivationFunctionType

    nc = bacc.Bacc()

    x_d = nc.dram_tensor("x_l", [BL, N, CIN], fp32, kind="ExternalInput")
    st_d = nc.dram_tensor("state_l", [BL, N, D], fp32, kind="ExternalInput")
    e_d = nc.dram_tensor("emb", [N, ED], fp32, kind="ExternalInput")
    wc_d = nc.dram_tensor("wc", [N, N], fp32, kind="ExternalInput")
    wd_d = nc.dram_tensor("wd", [N, N], fp32, kind="ExternalInput")
    gw_d = nc.dram_tensor("gw", [ED, 2, DIN, 2 * D], fp32, kind="ExternalInput")
    gb_d = nc.dram_tensor("gb", [ED, 2 * D], fp32, kind="ExternalInput")
    uw_d = nc.dram_tensor("uw", [ED, 2, DIN, D], fp32, kind="ExternalInput")
    ub_d = nc.dram_tensor("ub", [ED, D], fp32, kind="ExternalInput")
    out_d = nc.dram_tensor("out", [BL, N, D], fp32, kind="ExternalOutput")
    mc_d = nc.dram_tensor("maskc", [N, N], bf16, kind="Internal")
    md_d = nc.dram_tensor("maskd", [N, N], bf16, kind="Internal")

    with ExitStack() as ctx:
        tc = ctx.enter_context(tile.TileContext(nc))
        const = ctx.enter_context(tc.tile_pool(name="const", bufs=1))
        keep = ctx.enter_context(tc.tile_pool(name="keep", bufs=1))
        sup_pool = tc.tile_pool(name="sup", bufs=1)
        sup_ctx = sup_pool.__enter__()
        supT = sup_ctx.tile([P, NT, N], bf16, name="supT")

        ident = const.tile([P, P], fp32)
        make_identity(nc, ident)
        identb = const.tile([P, P], bf16)
        make_identity(nc, identb)
        ones_col = const.tile([P, 1], fp32)
        nc.vector.memset(ones_col, 1.0)
        ones_row = const.tile([1, P], fp32)
        nc.vector.memset(ones_row, 1.0)
        ones_colb = const.tile([P, 1], bf16)
        nc.vector.memset(ones_colb, 1.0)
        negfour = const.tile([P, 1], fp32)
        nc.vector.memset(negfour, -4.0)

        # ---- E natural + E^T -------------------------------------------------
        e_nat = const.tile([P, NT, ED], fp32)
        nc.sync.dma_start(out=e_nat, in_=e_d.rearrange("(t p) e -> p t e", p=P))
        eT = const.tile([ED, N], fp32)
        with tc.tile_pool(name="dummy", bufs=1, space="PSUM") as dpool:
            # absorber matmuls: pre-warm PE's vector clock on the Pool
            # (identity) and DMA (e_nat) semaphores so that the transpose
            # matmuls below need <=1 sync wait (S3_LW has one wait slot).
            dmy = dpool.tile([ED, P], fp32, name="dmy", tag="d1")
            nc.tensor.matmul(dmy, e_nat[:, 0, :], ident, start=True, stop=True)
            dmy2 = dpool.tile([P, P], fp32, name="dmy2", tag="d2")
            nc.tensor.matmul(dmy2, identb, identb, start=True, stop=True)
        with tc.tile_pool(name="ps0", bufs=2, space="PSUM") as ps0:
            for nt in range(NT):
                pt = ps0.tile([ED, P], fp32, tag="tp0")
                nc.tensor.transpose(pt, e_nat[:, nt, :], ident)
                nc.vector.tensor_copy(eT[:, nt * P:(nt + 1) * P], pt)

        # ---- biases b_g = E@gate_b, b_u = E@upd_b ---------------------------
        gb_st = const.tile([ED, 2 * D], fp32)
        nc.sync.dma_start(out=gb_st, in_=gb_d[:, :])
        ub_st = const.tile([ED, D], fp32)
        nc.sync.dma_start(out=ub_st, in_=ub_d[:, :])
        # staged through DVE so bias matmuls wait on a single semaphore
        gb_sb = const.tile([ED, 2 * D], fp32)
        nc.vector.tensor_copy(gb_sb, gb_st)
        ub_sb = const.tile([ED, D], fp32)
        nc.vector.tensor_copy(ub_sb, ub_st)
        bg = const.tile([P, NT, 2 * D], fp32)
        bu = const.tile([P, NT, D], fp32)
        with tc.tile_pool(name="psb", bufs=2, space="PSUM") as psb:
            for nt in range(NT):
                pb = psb.tile([P, 2 * D], fp32, tag="bias")
                nc.tensor.matmul(pb, eT[:, nt * P:(nt + 1) * P], gb_sb,
                                 start=True, stop=True)
                nc.vector.tensor_copy(bg[:, nt, :], pb)
                pb2 = psb.tile([P, D], fp32, tag="bias2")
                nc.tensor.matmul(pb2, eT[:, nt * P:(nt + 1) * P], ub_sb,
                                 start=True, stop=True)
                nc.vector.tensor_copy(bu[:, nt, :], pb2)

        # ---- weight pools rearranged to rhs form [i, (e o)] bf16 ------------
        wg = []
        wu = []
        for k in range(2):
            stg = const.tile([DIN, ED, 2 * D], fp32, name=f"stg_g{k}", tag=f"stg_g{k}")
            nc.sync.dma_start(out=stg, in_=gw_d[:, k, :, :].rearrange("e i o -> i e o"))
            wgk = const.tile([DIN, ED * 2 * D], bf16, name=f"wgk{k}", tag=f"wgk{k}")
            nc.vector.tensor_copy(wgk.rearrange("i (e o) -> i e o", e=ED), stg)
            wg.append(wgk)
        for k in range(2):
            stg = const.tile([DIN, ED, D], fp32, name=f"stg_u{k}", tag=f"stg_u{k}")
            nc.sync.dma_start(out=stg, in_=uw_d[:, k, :, :].rearrange("e i o -> i e o"))
            wuk = const.tile([DIN, ED * D], bf16, name=f"wuk{k}", tag=f"wuk{k}")
            nc.vector.tensor_copy(wuk.rearrange("i (e o) -> i e o", e=ED), stg)
            wu.append(wuk)

        # ---- A = E E^T: supports^T (normalized, bf16) + mask_c to DRAM ------
        with tc.tile_pool(name="p0m", bufs=3) as p0m, \
             tc.tile_pool(name="p0acc", bufs=1) as p0acc:
            with tc.tile_pool(name="p0a", bufs=2, space="PSUM") as p0a:
                for j in range(NT):
                    aj = p0a.tile([P, N], fp32, tag="A")
                    for c in range(4):
                        nc.tensor.matmul(aj[:, c * 512:(c + 1) * 512],
                                         eT[:, j * P:(j + 1) * P],
                                         eT[:, c * 512:(c + 1) * 512],
                                         start=True, stop=True)
                    mkj = p0m.tile([P, N], bf16, tag="mk")
                    nc.vector.tensor_scalar(mkj, aj, 0.0, None, op0=Alu.is_gt)
                    nc.gpsimd.dma_start(out=mc_d[j * P:(j + 1) * P, :], in_=mkj)
                    mdj = p0m.tile([P, N], bf16, tag="md")
                    nc.vector.tensor_scalar(mdj, aj, 0.0, None, op0=Alu.is_lt)
                    nc.gpsimd.dma_start(out=md_d[j * P:(j + 1) * P, :], in_=mdj)
                    # exp(relu(A)) = max(exp(A), 1)
                    nc.scalar.activation(supT[:, j, :], aj, Act.Exp)
                    nc.vector.tensor_scalar(supT[:, j, :], supT[:, j, :], 1.0,
                                            None, op0=Alu.max)
            pcs_cm = tc.tile_pool(name="pcs", bufs=1, space="PSUM")
            pcs = pcs_cm.__enter__()
            # colsum over partitions (PE accumulation) -> recip -> broadcast
            csum = pcs.tile([P, N], fp32, tag="cs")
            for c in range(4):
                for j in range(NT):
                    nc.tensor.matmul(csum[0:1, c * 512:(c + 1) * 512], ones_colb,
                                     supT[:, j, c * 512:(c + 1) * 512],
                                     start=(j == 0), stop=(j == NT - 1))
            recip = p0acc.tile([1, N], fp32)
            nc.vector.reciprocal(recip, csum[0:1, :])
            rb = pcs.tile([P, N], fp32, tag="cs")
            for c in range(4):
                nc.tensor.matmul(rb[:, c * 512:(c + 1) * 512], ones_row,
                                 recip[:, c * 512:(c + 1) * 512], start=True, stop=True)
            rbs = p0acc.tile([P, N], fp32, tag="rbs")
            nc.vector.tensor_copy(rbs, rb)
            for j in range(NT):
                nc.vector.tensor_mul(supT[:, j, :], supT[:, j, :], rbs)
            pcs_cm.__exit__(None, None, None)

        # ---- phase 1: GRU per batch ----------------------------------------
        h_f = []
        h_b = []
        hT_b = []
        for b in range(BL):
            h_f.append(keep.tile([P, NT, D], fp32, tag=f"hf{b}", name=f"hf{b}"))
            h_b.append(keep.tile([P, NT, D], bf16, tag=f"hb{b}", name=f"hb{b}"))
            hT_b.append(keep.tile([D, N], bf16, tag=f"ht{b}", name=f"ht{b}"))

        for b in range(BL):
            with tc.tile_pool(name=f"g{b}", bufs=1) as gp, \
                 tc.tile_pool(name=f"ptr{b}", bufs=2, space="PSUM") as ptr, \
                 tc.tile_pool(name=f"pg{b}", bufs=1, space="PSUM") as pg, \
                 tc.tile_pool(name=f"py{b}", bufs=2, space="PSUM") as py:
                inp = gp.tile([P, NT, DIN], fp32)
                nc.sync.dma_start(out=inp[:, :, 0:CIN],
                                  in_=x_d[b].rearrange("(t p) c -> p t c", p=P))
                nc.sync.dma_start(out=inp[:, :, CIN:DIN],
                                  in_=st_d[b].rearrange("(t p) c -> p t c", p=P))
                inp_bf = gp.tile([P, NT, DIN], bf16)
                nc.vector.tensor_copy(inp_bf, inp)
                # inp^T bf16
                inpT = gp.tile([DIN, N], bf16)
                for nt in range(NT):
                    pt = ptr.tile([DIN, P], bf16, tag="tp")
                    nc.tensor.transpose(pt, inp_bf[:, nt, :], identb)
                    nc.vector.tensor_copy(inpT[:, nt * P:(nt + 1) * P], pt)
                # xg1^T = (sup @ inp)^T = inp(m-part) x supT
                xg1T = gp.tile([DIN, N], bf16)
                for half in range(2):
                    px = pg.tile([DIN, 1024], fp32, tag="xg")
                    for q in range(2):
                        for j in range(NT):
                            nc.tensor.matmul(
                                px[:, q * 512:(q + 1) * 512], inp_bf[:, j, :],
                                supT[:, j, half * 1024 + q * 512:
                                     half * 1024 + (q + 1) * 512],
                                start=(j == 0), stop=(j == NT - 1))
                    nc.vector.tensor_copy(xg1T[:, half * 1024:(half + 1) * 1024], px)
                # gate: y = xg0^T.T @ Wg0 + xg1^T.T @ Wg1  -> z_r
                zr = gp.tile([P, NT, 2 * D], fp32)
                for nt in range(NT):
                    lhs0 = inpT[:, nt * P:(nt + 1) * P]
                    lhs1 = xg1T[:, nt * P:(nt + 1) * P]
                    for c in range(2):
                        yg = py.tile([P, 1024], fp32, tag="y")
                        for q in range(2):
                            sl = slice(c * 1024 + q * 512, c * 1024 + (q + 1) * 512)
                            nc.tensor.matmul(yg[:, q * 512:(q + 1) * 512], lhs0,
                                             wg[0][:, sl], start=True, stop=False)
                            nc.tensor.matmul(yg[:, q * 512:(q + 1) * 512], lhs1,
                                             wg[1][:, sl], start=False, stop=True)
                        for ee in range(8):
                            e = c * 8 + ee
                            prev = bg[:, nt, :] if e == 0 else zr[:, nt, :]
                            nc.vector.scalar_tensor_tensor(
                                out=zr[:, nt, :], in0=yg[:, ee * 2 * D:(ee + 1) * 2 * D],
                                scalar=e_nat[:, nt, e:e + 1], in1=prev,
                                op0=Alu.mult, op1=Alu.add)
                    nc.scalar.activation(zr[:, nt, :], zr[:, nt, :], Act.Sigmoid)
                # candidate
                cand_bf = gp.tile([P, NT, DIN], bf16)
                nc.vector.tensor_copy(cand_bf[:, :, 0:CIN], inp[:, :, 0:CIN])
                for nt in range(NT):
                    nc.vector.tensor_mul(cand_bf[:, nt, CIN:DIN], zr[:, nt, 0:D],
                                         inp[:, nt, CIN:DIN])
                candT = gp.tile([DIN, N], bf16)
                for nt in range(NT):
                    pt = ptr.tile([DIN, P], bf16, tag="tp")
                    nc.tensor.transpose(pt, cand_bf[:, nt, :], identb)
                    nc.vector.tensor_copy(candT[:, nt * P:(nt + 1) * P], pt)
                xg1cT = gp.tile([DIN, N], bf16)
                for half in range(2):
                    px = pg.tile([DIN, 1024], fp32, tag="xg")
                    for q in range(2):
                        for j in range(NT):
                            nc.tensor.matmul(
                                px[:, q * 512:(q + 1) * 512], cand_bf[:, j, :],
                                supT[:, j, half * 1024 + q * 512:
                                     half * 1024 + (q + 1) * 512],
                                start=(j == 0), stop=(j == NT - 1))
                    nc.vector.tensor_copy(xg1cT[:, half * 1024:(half + 1) * 1024], px)
                # update sagc -> hc -> h
                for nt in range(NT):
                    lhs0 = candT[:, nt * P:(nt + 1) * P]
                    lhs1 = xg1cT[:, nt * P:(nt + 1) * P]
                    yu = py.tile([P, ED * D], fp32, tag="y")
                    for q in range(2):
                        sl = slice(q * 512, (q + 1) * 512)
                        nc.tensor.matmul(yu[:, sl], lhs0, wu[0][:, sl],
                                         start=True, stop=False)
                        nc.tensor.matmul(yu[:, sl], lhs1, wu[1][:, sl],
                                         start=False, stop=True)
                    hc = gp.tile([P, D], fp32, tag="hc")
                    for e in range(ED):
                        prev = bu[:, nt, :] if e == 0 else hc
                        nc.vector.scalar_tensor_tensor(
                            out=hc, in0=yu[:, e * D:(e + 1) * D],
                            scalar=e_nat[:, nt, e:e + 1], in1=prev,
                            op0=Alu.mult, op1=Alu.add)
                    nc.scalar.activation(hc, hc, Act.Tanh)
                    # h = r*state + (1-r)*hc = hc + r*(state-hc)
                    tdiff = gp.tile([P, D], fp32, tag="td")
                    nc.vector.tensor_sub(tdiff, inp[:, nt, CIN:DIN], hc)
                    nc.vector.tensor_mul(tdiff, tdiff, zr[:, nt, D:2 * D])
                    nc.vector.tensor_add(h_f[b][:, nt, :], tdiff, hc)
                nc.vector.tensor_copy(h_b[b], h_f[b])
                for nt in range(NT):
                    pt = ptr.tile([D, P], bf16, tag="tp")
                    nc.tensor.transpose(pt, h_b[b][:, nt, :], identb)
                    nc.vector.tensor_copy(hT_b[b][:, nt * P:(nt + 1) * P], pt)

        sup_pool.__exit__(None, None, None)

        # ---- phase 2: degrees ------------------------------------------------
        with tc.tile_pool(name="mbig", bufs=1) as mbig, \
             tc.tile_pool(name="p2s", bufs=2) as p2s, \
             tc.tile_pool(name="p2acc", bufs=1) as p2acc:
            for b in range(BL):
                degs = []
                for pi, w_dram in enumerate([wc_d, wd_d]):
                    Mb = mbig.tile([P, NT, N], bf16, tag="Mb")
                    with tc.tile_pool(name="hhtp", bufs=2, space="PSUM") as hhtp:
                        for j in range(NT):
                            hht = hhtp.tile([P, N], fp32, tag="hht")
                            for c in range(4):
                                nc.tensor.matmul(
                                    hht[:, c * 512:(c + 1) * 512],
                                    hT_b[b][:, j * P:(j + 1) * P],
                                    hT_b[b][:, c * 512:(c + 1) * 512],
                                    start=True, stop=True)
                            mk = p2s.tile([P, N], bf16, tag="mk", bufs=3)
                            msk_d = mc_d if pi == 0 else md_d
                            nc.sync.dma_start(out=mk, in_=msk_d[j * P:(j + 1) * P, :])
                            nc.vector.tensor_mul(Mb[:, j, :], mk, hht)
                    acc2 = p2acc.tile([P, N], fp32, tag="acc2")
                    nc.vector.memset(acc2, 0.0)
                    outT_sb = p2acc.tile([D, N], fp32, tag="oT",
                                          name=f"outT{b}{pi}")
                    with tc.tile_pool(name="rp", bufs=2, space="PSUM") as rp, \
                         tc.tile_pool(name="op", bufs=1, space="PSUM") as op:
                        # out^T accumulated as [64, 2048]: one 512-wide region
                        # per PSUM bank per accumulation group (start=True
                        # clears has_written bank-wide, so regions must not
                        # share banks)
                        outacc = op.tile([D, N], fp32, tag="oa")
                        for kt in range(NT):
                            wb = p2s.tile([P, NT, P], bf16, tag="wb")
                            for j in range(NT):
                                wf = p2s.tile([P, P], fp32, tag="wf", bufs=4)
                                nc.sync.dma_start(
                                    out=wf,
                                    in_=w_dram[j * P:(j + 1) * P,
                                               kt * P:(kt + 1) * P])
                                nc.vector.tensor_copy(wb[:, j, :], wf)
                            PT = p2s.tile([P, N], bf16, tag="PT", bufs=3)
                            for half in range(2):
                                R = rp.tile([P, 1024], fp32, tag="R")
                                for q in range(2):
                                    for j in range(NT):
                                        nc.tensor.matmul(
                                            R[:, q * 512:(q + 1) * 512], wb[:, j, :],
                                            Mb[:, j, half * 1024 + q * 512:
                                               half * 1024 + (q + 1) * 512],
                                            start=(j == 0), stop=(j == NT - 1))
                                # bias -4 keeps unnormalized exp in fp16 range;
                                # cancels exactly in the softmax normalization
                                nc.scalar.activation(
                                    PT[:, half * 1024:(half + 1) * 1024], R, Act.Exp,
                                    bias=negfour)
                                nc.vector.tensor_add(
                                    acc2[:, half * 1024:(half + 1) * 1024],
                                    acc2[:, half * 1024:(half + 1) * 1024],
                                    PT[:, half * 1024:(half + 1) * 1024])
                            for c in range(4):
                                nc.tensor.matmul(outacc[:, c * 512:(c + 1) * 512],
                                                 h_b[b][:, kt, :],
                                                 PT[:, c * 512:(c + 1) * 512],
                                                 start=(kt == 0), stop=(kt == NT - 1))
                        nc.vector.tensor_copy(outT_sb, outacc)
                    with tc.tile_pool(name="csp", bufs=2, space="PSUM") as csp, \
                         tc.tile_pool(name="ptp", bufs=2, space="PSUM") as ptp:
                        rcol = p2acc.tile([P, NT], fp32, tag="rcol")
                        for nt in range(NT):
                            cs = csp.tile([P, 1], fp32, tag="cs")
                            nc.tensor.matmul(cs, acc2[:, nt * P:(nt + 1) * P],
                                             ones_col, start=True, stop=True)
                            nc.vector.reciprocal(rcol[:, nt:nt + 1], cs)
                        deg = p2acc.tile([P, NT, D], fp32, tag=f"deg{pi}")
                        for nt in range(NT):
                            po = ptp.tile([P, D], fp32, tag="po")
                            nc.tensor.transpose(
                                po, outT_sb[:, nt * P:(nt + 1) * P],
                                ident[0:D, 0:D])
                            nc.vector.tensor_scalar(deg[:, nt, :], po,
                                                    rcol[:, nt:nt + 1], None,
                                                    op0=Alu.mult)
                        degs.append(deg)
                # combine: 0.8 h - 0.1 deg_c + 0.1 deg_d
                res = p2acc.tile([P, NT, D], fp32, tag="res")
                for nt in range(NT):
                    nc.vector.tensor_scalar(res[:, nt, :], h_f[b][:, nt, :], 0.8,
                                            None, op0=Alu.mult)
                    nc.vector.scalar_tensor_tensor(
                        out=res[:, nt, :], in0=degs[0][:, nt, :], scalar=-0.1,
                        in1=res[:, nt, :], op0=Alu.mult, op1=Alu.add)
                    nc.vector.scalar_tensor_tensor(
                        out=res[:, nt, :], in0=degs[1][:, nt, :], scalar=0.1,
                        in1=res[:, nt, :], op0=Alu.mult, op1=Alu.add)
                nc.sync.dma_start(out=out_d[b].rearrange("(t p) d -> p t d", p=P),
                                  in_=res)
    nc.compile()
    return nc


_CACHED = {}


def kernel(x, state, node_embeddings, weight_connect, weight_disconnect,
           gate_w, gate_b, upd_w, upd_b, _trace=False):
    from concourse.bass_utils import run_bass_kernel_spmd

    if "nc" not in _CACHED:
        _CACHED["nc"] = _build_program()
    nc = _CACHED["nc"]

    shared = {
        "emb": np.ascontiguousarray(node_embeddings, dtype=np.float32),
        "wc": np.ascontiguousarray(weight_connect, dtype=np.float32),
        "wd": np.ascontiguousarray(weight_disconnect, dtype=np.float32),
        "gw": np.ascontiguousarray(gate_w, dtype=np.float32),
        "gb": np.ascontiguousarray(gate_b, dtype=np.float32),
        "uw": np.ascontiguousarray(upd_w, dtype=np.float32),
        "ub": np.ascontiguousarray(upd_b, dtype=np.float32),
    }
    in_maps = []
    for c in range(NCORES):
        m = dict(shared)
        m["x_l"] = np.ascontiguousarray(x[c * BL:(c + 1) * BL], dtype=np.float32)
        m["state_l"] = np.ascontiguousarray(state[c * BL:(c + 1) * BL],
                                            dtype=np.float32)
        in_maps.append(m)

    res = run_bass_kernel_spmd(nc, in_maps, core_ids=list(range(NCORES)),
                               trace=_trace)
    out = np.concatenate([r["out"] for r in res.results], axis=0)
    if _trace:
        kernel._last_results = res
    return out
